# revision 22
# baseline (speedup 1.0000x reference)
"""Trainium2 Bass kernel for single-head attention with softmax over the query axis.

Reference computation (B=4, S=4096, DIM=768, D=96):
    q = x @ Wq + bq; k = x @ Wk + bk; v = x @ Wv + bv        # [B,S,D]
    att = einsum('bqd,bkd->bqk', q, k) / sqrt(D)             # [B,Sq,Sk]
    p   = softmax(att, axis=1)                               # over the QUERY axis
    out = einsum('bqk,bkd->bqd', p, v)

Sharding: 8 cores = 4 batches x 2 key-halves. Softmax over q is local to a
key-shard (it normalizes each key-column over all queries), and the output
contraction over k is a sum over the two key-halves, done host-side.

SPMD uniformity trick: every core runs the identical program "K/V come from
rows 0:2048 of my x, Q from all 4096 rows". The host hands core (b, kh=1) a
row-rolled copy of x[b] so its key half lands in rows 0:2048; softmax over q
is permutation-invariant, and the host un-rolls that core's partial output.

Host precomputation (legal data prep inside kernel()): x is rolled,
transposed to xT [768, 4096] and cast to fp16; Wq/bq are pre-scaled by
1/sqrt(D) so no separate score scaling is needed; weights pre-cast to fp16.

On-device (per core):
  xT  [768, 4096] fp16 in SBUF (12 KB/partition)
  QT = Wq^T xT  [96, 4096], KT/VT likewise for rows 0:2048      (fp16)
  V[kk]  [128, 96] = PE-transpose of VT 128-column blocks        (fp16)
  scoresT[128k, q] = KT_kk^T QT in PSUM; exp on scalar engine with fused
      row-sum (accum_out); no max-subtraction (scores bounded ~|7.3|)
  Vs[kk] = V[kk] * (1/rowsum_kk)  folds softmax normalization into V
  outT[96, 4096] += Vs_kk^T @ expT_kk, accumulated in PSUM over kk;
      PV for q-blocks 0:2048 is software-pipelined inside the scores/exp
      loop (PSUM: 4 banks scores + 4 banks PV), the rest follows after.
"""

import os
import sys

import numpy as np

for _p in ("/opt/trn_rl_repo",):
    if _p not in sys.path and os.path.isdir(_p):
        sys.path.insert(0, _p)

B, S, DIM, D = 4, 4096, 768, 96
SK = S // 2          # local keys per core
N_CORES = 8
NDC = DIM // 128     # 6 dim chunks
NKK = SK // 128      # 16 local key chunks

_CACHE = {}


def _build_module():
    import concourse.bass as bass
    import concourse.tile as tile
    from concourse import bacc, mybir
    from concourse.masks import make_identity
    from concourse.tile import add_dep_helper

    fp32 = mybir.dt.float32
    fp16 = mybir.dt.float16

    nc = bacc.Bacc("TRN2", target_bir_lowering=False, debug=False,
                   num_devices=N_CORES)

    xT_ap = nc.dram_tensor("xT", [DIM, S], fp16, kind="ExternalInput").ap()
    wq_ap = nc.dram_tensor("wq", [DIM, D], fp16, kind="ExternalInput").ap()
    wk_ap = nc.dram_tensor("wk", [DIM, D], fp16, kind="ExternalInput").ap()
    wv_ap = nc.dram_tensor("wv", [DIM, D], fp16, kind="ExternalInput").ap()
    bq_ap = nc.dram_tensor("bq", [D, 1], fp32, kind="ExternalInput").ap()
    bk_ap = nc.dram_tensor("bk", [D, 1], fp32, kind="ExternalInput").ap()
    bv_ap = nc.dram_tensor("bv", [D, 1], fp32, kind="ExternalInput").ap()
    outT_ap = nc.dram_tensor("outT", [D, S], fp16, kind="ExternalOutput").ap()

    with tile.TileContext(nc) as tc:
        with (
            tc.tile_pool(name="singles", bufs=1) as singles,
            tc.tile_pool(name="acts", bufs=1) as acts,
            tc.tile_pool(name="outp", bufs=4) as outp,
        ):
            # Weights/biases first (tiny), then xT halves chained two-deep
            # so early chunks finish early (concurrent DMAs share bandwidth
            # equally; unchained, the first chunk lands no earlier than the
            # last and compute can't start for ~17us).
            w_sb = {}
            for name, ap in (("q", wq_ap), ("k", wk_ap), ("v", wv_ap)):
                w = singles.tile([128, NDC, D], fp16, tag=f"w{name}")
                nc.gpsimd.dma_start(w[:], ap.rearrange("(c p) j -> p c j", p=128))
                w_sb[name] = w
            b_sb = {}
            for name, ap in (("q", bq_ap), ("k", bk_ap), ("v", bv_ap)):
                t = singles.tile([D, 1], fp32, tag=f"b{name}")
                nc.gpsimd.dma_start(t[:], ap[:])
                b_sb[name] = t
            xTs = []
            for dc in range(NDC):
                t = singles.tile([128, S], fp16, tag=f"xT{dc}",
                                 name=f"xT{dc}")
                xTs.append(t)
            # Head pieces land first so QT-sb0/sb1 and KT-sb0/1 can start
            # immediately; later pieces chained behind so the heads get the
            # bandwidth. head1 descriptors issue from the scalar engine in
            # parallel with sync issuing the rest.
            head1, head2, tail, hi_d = [], [], [], []
            for dc in range(NDC):
                head1.append(nc.scalar.dma_start(
                    xTs[dc][:, :512],
                    xT_ap[dc * 128:(dc + 1) * 128, :512]))
            for dc in range(NDC):
                head2.append(nc.sync.dma_start(
                    xTs[dc][:, 512:1024],
                    xT_ap[dc * 128:(dc + 1) * 128, 512:1024]))
            for dc in range(NDC):
                tail.append(nc.sync.dma_start(
                    xTs[dc][:, 1024:SK],
                    xT_ap[dc * 128:(dc + 1) * 128, 1024:SK]))
            for dc in range(NDC):
                hi_d.append(nc.sync.dma_start(
                    xTs[dc][:, SK:],
                    xT_ap[dc * 128:(dc + 1) * 128, SK:]))
            for dc in range(NDC):
                add_dep_helper(tail[dc].ins, head2[dc].ins,
                               reason="xT lo tails yield bandwidth to heads")
                add_dep_helper(hi_d[dc].ins, tail[dc].ins,
                               reason="xT hi halves yield bandwidth to lo")
            identity = singles.tile([128, 128], fp16)
            make_identity(nc, identity[:])

            QT = acts.tile([D, S], fp16, tag="QT")
            KT = acts.tile([D, SK], fp16, tag="KT")
            VT = acts.tile([D, SK], fp16, tag="VT")
            V = acts.tile([128, NKK, D], fp16, tag="V")
            Vs = acts.tile([128, NKK, D], fp16, tag="Vs")
            # S-A (2048-wide) writes slot 0 only (kk=0: slots 0,1), S-B
            # writes slots 2,3; zero-fill so the rsum reduce over all 4
            # slots is correct for every kk.
            sums = acts.tile([128, NKK, 4], fp32, tag="sums")
            nc.vector.memset(sums[:], 0.0)
            rsum = acts.tile([128, NKK], fp32, tag="rsum")
            rrec = acts.tile([128, NKK], fp32, tag="rrec")
            sum0b = acts.tile([128, 1], fp32, tag="sum0b")
            # exp(scores) for q 0:2048 (written by S-A) stays resident for
            # the trailing PV pass; q 2048:4096 rotates through a small pool
            # consumed by the PV pipelined inside S-B.
            expT_A = acts.tile([128, NKK, S // 2], fp16, tag="expT_A")

            # ---------------- Phases -------------------------------------
            # P1-head: QT-sb0 + KT-sb0 (gated only on the first head DMA
            #     pieces) so exp starts as early as possible.
            # S-A: scores+exp for q 0:2048 into the persistent expT_A at
            #     [128,2048] exp granularity (the PV pool isn't open yet so
            #     scores can take 4 PSUM banks x 2 buffers = all 8). All
            #     remaining projection work runs as self-contained units,
            #     one per S-A step, allocating transient accumulators from
            #     the same rotating pool.
            # S-B: scores+exp for q 2048:4096 at [128,1024] granularity
            #     (4 banks) + PV for q 2048:4096 accumulating in the other
            #     4 banks, software-pipelined one kk behind.
            # O2: PV for q 0:2048 from expT_A.
            def pv_matmuls(kk, po, src_tile, src_off):
                for qb in range(4):
                    nc.tensor.matmul(
                        po[qb][:], Vs[:, kk, :],
                        src_tile[:, src_off + qb * 512:
                                 src_off + (qb + 1) * 512],
                        start=(kk == 0), stop=(kk == NKK - 1))

            def drain_po(po, qb_base):
                for qb in range(4):
                    ob = outp.tile([D, 512], fp16, tag="ob")
                    nc.vector.tensor_copy(ob[:], po[qb][:])
                    nc.sync.dma_start(
                        outT_ap[:, (qb_base + qb) * 512:
                                (qb_base + qb + 1) * 512], ob[:])

            with tc.tile_pool(name="ps_sA", bufs=4, space="PSUM") as ps_sA:

                def proj_block(wname, dst, bias, sb):
                    acc = ps_sA.tile([D, 512], fp32, tag="ps",
                                     name=f"a{wname}{sb}")
                    for dc in range(NDC):
                        nc.tensor.matmul(
                            acc[:], w_sb[wname][:, dc, :],
                            xTs[dc][:, sb * 512:(sb + 1) * 512],
                            start=(dc == 0), stop=(dc == NDC - 1))
                    nc.vector.tensor_scalar_add(
                        dst[:, sb * 512:(sb + 1) * 512], acc[:], bias[:])

                def v_trans4(kk4):
                    pt = ps_sA.tile([128, 4, D], fp16, tag="ps",
                                    name=f"pt{kk4}")
                    for k in range(4):
                        nc.tensor.transpose(
                            pt[:, k, :],
                            VT[:, (kk4 + k) * 128:(kk4 + k + 1) * 128],
                            identity[:D, :D])
                    nc.vector.tensor_copy(V[:, kk4:kk4 + 4, :], pt[:])

                # P1 head: only what the first scores need.
                proj_block("q", QT, b_sb["q"], 0)
                proj_block("k", KT, b_sb["k"], 0)

                # Deferred units, one per S-A step, ordered by DMA arrival:
                # lo tails (~20us) before hi halves (~28us); V-transposes
                # (no DMA dependency) last.
                units = [
                    lambda: proj_block("k", KT, b_sb["k"], 2),
                    lambda: proj_block("k", KT, b_sb["k"], 3),
                    lambda: proj_block("q", QT, b_sb["q"], 2),
                    lambda: proj_block("q", QT, b_sb["q"], 3),
                    lambda: proj_block("v", VT, b_sb["v"], 0),
                    lambda: proj_block("v", VT, b_sb["v"], 1),
                    lambda: proj_block("v", VT, b_sb["v"], 2),
                    lambda: proj_block("v", VT, b_sb["v"], 3),
                    lambda: proj_block("q", QT, b_sb["q"], 4),
                    lambda: proj_block("q", QT, b_sb["q"], 5),
                    lambda: proj_block("q", QT, b_sb["q"], 6),
                    lambda: proj_block("q", QT, b_sb["q"], 7),
                    lambda: v_trans4(0),
                    lambda: v_trans4(4),
                    lambda: v_trans4(8),
                    lambda: v_trans4(12),
                ]

                # S-A: qq-outer so the whole first pass (q 0:1024) runs off
                # the head DMA pieces alone.
                ui = 0
                for qq in range(2):
                    for kk in range(NKK):
                        if qq == 0 and kk == 0:
                            # Finer first steps off the first head piece,
                            # second-head projections interleaved.
                            for j, acc in ((0, sums[:, 0, 0:1]),
                                           (1, sum0b[:])):
                                ps = ps_sA.tile([128, 512], fp32,
                                                tag="ps", name=f"ps0{j}")
                                nc.tensor.matmul(
                                    ps[:], KT[:, :128],
                                    QT[:, j * 512:(j + 1) * 512],
                                    start=True, stop=True)
                                nc.scalar.activation(
                                    expT_A[:, 0, j * 512:(j + 1) * 512],
                                    ps[:],
                                    mybir.ActivationFunctionType.Exp,
                                    accum_out=acc)
                                if j == 0:
                                    proj_block("q", QT, b_sb["q"], 1)
                                    proj_block("k", KT, b_sb["k"], 1)
                            continue
                        ps = ps_sA.tile([128, 1024], fp32, tag="ps")
                        for j in range(2):
                            nc.tensor.matmul(
                                ps[:, j * 512:(j + 1) * 512],
                                KT[:, kk * 128:(kk + 1) * 128],
                                QT[:, qq * 1024 + j * 512:
                                   qq * 1024 + (j + 1) * 512],
                                start=True, stop=True)
                        nc.scalar.activation(
                            expT_A[:, kk, qq * 1024:(qq + 1) * 1024],
                            ps[:], mybir.ActivationFunctionType.Exp,
                            accum_out=sums[:, kk, qq:qq + 1])
                        if ui < len(units):
                            units[ui]()
                            ui += 1
                while ui < len(units):
                    units[ui]()
                    ui += 1

            # S-B: scores+exp for q 2048:4096 + pipelined PV(q hi half).
            with (
                tc.tile_pool(name="ps_sB", bufs=2, space="PSUM") as ps_sB,
                tc.tile_pool(name="ps_o1", bufs=4, space="PSUM") as ps_o1,
                tc.tile_pool(name="exphi", bufs=2) as exphi_pool,
            ):
                po1 = [ps_o1.tile([D, 512], fp32, tag="po",
                                  name=f"po1_{i}") for i in range(4)]
                prev_hi = None
                for kk in range(NKK):
                    exp_hi = exphi_pool.tile([128, S // 2], fp16,
                                             tag="exp_hi")
                    for qq in (2, 3):
                        ps = ps_sB.tile([128, 1024], fp32, tag="psb")
                        for j in range(2):
                            nc.tensor.matmul(
                                ps[:, j * 512:(j + 1) * 512],
                                KT[:, kk * 128:(kk + 1) * 128],
                                QT[:, qq * 1024 + j * 512:
                                   qq * 1024 + (j + 1) * 512],
                                start=True, stop=True)
                        nc.scalar.activation(
                            exp_hi[:, (qq - 2) * 1024:(qq - 1) * 1024],
                            ps[:], mybir.ActivationFunctionType.Exp,
                            accum_out=sums[:, kk, qq:qq + 1])
                    nc.vector.reduce_sum(rsum[:, kk:kk + 1],
                                         sums[:, kk, :],
                                         axis=mybir.AxisListType.X)
                    if kk == 0:
                        nc.vector.tensor_add(rsum[:, 0:1], rsum[:, 0:1],
                                             sum0b[:])
                    nc.vector.reciprocal(rrec[:, kk:kk + 1],
                                         rsum[:, kk:kk + 1])
                    nc.vector.tensor_scalar_mul(Vs[:, kk, :], V[:, kk, :],
                                                rrec[:, kk:kk + 1])
                    if kk > 0:
                        pv_matmuls(kk - 1, po1, prev_hi[:], 0)
                    prev_hi = exp_hi
                pv_matmuls(NKK - 1, po1, prev_hi[:], 0)
                drain_po(po1, 4)

            # O2: PV for q 0:2048 from the persistent expT_A.
            with tc.tile_pool(name="ps_o2", bufs=4, space="PSUM") as ps_o2:
                po2 = [ps_o2.tile([D, 512], fp32, tag="po2",
                                  name=f"po2_{i}") for i in range(4)]
                for kk in range(NKK - 1):
                    pv_matmuls(kk, po2, expT_A[:, kk, :], 0)
                for qb in range(4):
                    nc.tensor.matmul(
                        po2[qb][:], Vs[:, NKK - 1, :],
                        expT_A[:, NKK - 1, qb * 512:(qb + 1) * 512],
                        start=False, stop=True)
                    ob = outp.tile([D, 512], fp16, tag="ob")
                    (nc.vector.tensor_copy if qb % 2 == 0
                     else nc.scalar.copy)(ob[:], po2[qb][:])
                    nc.sync.dma_start(
                        outT_ap[:, qb * 512:(qb + 1) * 512], ob[:])

    _dedup_ldweights(nc, mybir)
    nc.compile()
    return nc


def _dedup_ldweights(nc, mybir):
    """Drop InstLdweights that reload the weights already resident in the PE
    array (identical source AP as the previous load, with only
    non-self-loading matmuls in between). Tile's lowering emits one
    LDWEIGHTS per matmul; consecutive matmuls sharing a stationary operand
    only need the first."""
    remap = {}
    removed = 0
    for fn in nc.m.functions:
        for bb in fn.blocks:
            keep = []
            last_sig = None
            last_kept = None
            for inst in bb.instructions:
                if isinstance(inst, mybir.InstLdweights):
                    w = inst.ins[0]
                    try:
                        sig = (str(w.memref), str(w.memsetref), w.offset,
                               str(w.ap), str(w.dtype),
                               inst.perf_mode, inst.is_transpose)
                    except Exception:
                        sig = None
                    if sig is not None and last_kept is not None \
                            and sig == last_sig:
                        remap[inst.name] = last_kept.name
                        del nc.inst_map[inst.name]
                        removed += 1
                        continue
                    last_sig = sig
                    last_kept = inst
                elif isinstance(inst, mybir.InstMatmult):
                    if inst.ldweights is not False:
                        last_sig = None
                        last_kept = None
                keep.append(inst)
            if len(keep) != len(bb.instructions):
                bb.instructions[:] = keep
    if remap:
        for fn in nc.m.functions:
            for bb in fn.blocks:
                for inst in bb.instructions:
                    inst.remap_dependency_names(remap)
    return removed


def _get_module():
    if "nc" not in _CACHE:
        _CACHE["nc"] = _build_module()
    return _CACHE["nc"]


def kernel(x, Wq, bq, Wk, bk, Wv, bv, _trace=False):
    from concourse.bass_utils import run_bass_kernel_spmd

    x = np.asarray(x, dtype=np.float32)
    Wq = np.asarray(Wq, dtype=np.float32)
    bq = np.asarray(bq, dtype=np.float32)
    Wk = np.asarray(Wk, dtype=np.float32)
    bk = np.asarray(bk, dtype=np.float32)
    Wv = np.asarray(Wv, dtype=np.float32)
    bv = np.asarray(bv, dtype=np.float32)

    nc = _get_module()

    scale = np.float32(1.0 / np.sqrt(D))
    wq16 = (Wq * scale).astype(np.float16)
    wk16 = Wk.astype(np.float16)
    wv16 = Wv.astype(np.float16)
    bq_s = (bq * scale).astype(np.float32).reshape(D, 1)
    bk_s = bk.astype(np.float32).reshape(D, 1)
    bv_s = bv.astype(np.float32).reshape(D, 1)

    in_maps = []
    for c in range(N_CORES):
        b, kh = divmod(c, 2)
        xb = x[b]
        if kh:
            xb = np.concatenate([xb[SK:], xb[:SK]], axis=0)
        in_maps.append({
            "xT": np.ascontiguousarray(xb.T).astype(np.float16),
            "wq": wq16, "wk": wk16, "wv": wv16,
            "bq": bq_s, "bk": bk_s, "bv": bv_s,
        })

    res = run_bass_kernel_spmd(nc, in_maps,
                               core_ids=list(range(N_CORES)), trace=_trace)

    out = np.zeros((B, S, D), dtype=np.float32)
    for c in range(N_CORES):
        b, kh = divmod(c, 2)
        o = res.results[c]["outT"].T.astype(np.float32)  # [S, D], rolled q-order
        if kh:
            o = np.concatenate([o[SK:], o[:SK]], axis=0)
        out[b] += o
    if _trace:
        kernel.last_exec_time_ns = res.exec_time_ns
        kernel.last_result = res
    return out


# revision 23
# speedup vs baseline: 1.0018x; 1.0018x over previous
"""Trainium2 Bass kernel for single-head attention with softmax over the query axis.

Reference computation (B=4, S=4096, DIM=768, D=96):
    q = x @ Wq + bq; k = x @ Wk + bk; v = x @ Wv + bv        # [B,S,D]
    att = einsum('bqd,bkd->bqk', q, k) / sqrt(D)             # [B,Sq,Sk]
    p   = softmax(att, axis=1)                               # over the QUERY axis
    out = einsum('bqk,bkd->bqd', p, v)

Sharding: 8 cores = 4 batches x 2 key-halves. Softmax over q is local to a
key-shard (it normalizes each key-column over all queries), and the output
contraction over k is a sum over the two key-halves, done host-side.

SPMD uniformity trick: every core runs the identical program "K/V come from
rows 0:2048 of my x, Q from all 4096 rows". The host hands core (b, kh=1) a
row-rolled copy of x[b] so its key half lands in rows 0:2048; softmax over q
is permutation-invariant, and the host un-rolls that core's partial output.

Host precomputation (legal data prep inside kernel()): x is rolled,
transposed to xT [768, 4096] and cast to fp16; Wq/bq are pre-scaled by
1/sqrt(D) so no separate score scaling is needed; weights pre-cast to fp16.

On-device (per core):
  xT  [768, 4096] fp16 in SBUF (12 KB/partition)
  QT = Wq^T xT  [96, 4096], KT/VT likewise for rows 0:2048      (fp16)
  V[kk]  [128, 96] = PE-transpose of VT 128-column blocks        (fp16)
  scoresT[128k, q] = KT_kk^T QT in PSUM; exp on scalar engine with fused
      row-sum (accum_out); no max-subtraction (scores bounded ~|7.3|)
  Vs[kk] = V[kk] * (1/rowsum_kk)  folds softmax normalization into V
  outT[96, 4096] += Vs_kk^T @ expT_kk, accumulated in PSUM over kk;
      PV for q-blocks 0:2048 is software-pipelined inside the scores/exp
      loop (PSUM: 4 banks scores + 4 banks PV), the rest follows after.
"""

import os
import sys

import numpy as np

for _p in ("/opt/trn_rl_repo",):
    if _p not in sys.path and os.path.isdir(_p):
        sys.path.insert(0, _p)

B, S, DIM, D = 4, 4096, 768, 96
SK = S // 2          # local keys per core
N_CORES = 8
NDC = DIM // 128     # 6 dim chunks
NKK = SK // 128      # 16 local key chunks

_CACHE = {}


def _build_module():
    import concourse.bass as bass
    import concourse.tile as tile
    from concourse import bacc, mybir
    from concourse.masks import make_identity
    from concourse.tile import add_dep_helper

    fp32 = mybir.dt.float32
    fp16 = mybir.dt.float16

    nc = bacc.Bacc("TRN2", target_bir_lowering=False, debug=False,
                   num_devices=N_CORES)

    xT_ap = nc.dram_tensor("xT", [DIM, S], fp16, kind="ExternalInput").ap()
    wq_ap = nc.dram_tensor("wq", [DIM, D], fp16, kind="ExternalInput").ap()
    wk_ap = nc.dram_tensor("wk", [DIM, D], fp16, kind="ExternalInput").ap()
    wv_ap = nc.dram_tensor("wv", [DIM, D], fp16, kind="ExternalInput").ap()
    bq_ap = nc.dram_tensor("bq", [D, 1], fp32, kind="ExternalInput").ap()
    bk_ap = nc.dram_tensor("bk", [D, 1], fp32, kind="ExternalInput").ap()
    bv_ap = nc.dram_tensor("bv", [D, 1], fp32, kind="ExternalInput").ap()
    outT_ap = nc.dram_tensor("outT", [D, S], fp16, kind="ExternalOutput").ap()

    with tile.TileContext(nc) as tc:
        with (
            tc.tile_pool(name="singles", bufs=1) as singles,
            tc.tile_pool(name="acts", bufs=1) as acts,
            tc.tile_pool(name="outp", bufs=4) as outp,
        ):
            # Weights/biases first (tiny), then xT halves chained two-deep
            # so early chunks finish early (concurrent DMAs share bandwidth
            # equally; unchained, the first chunk lands no earlier than the
            # last and compute can't start for ~17us).
            w_sb = {}
            for name, ap in (("q", wq_ap), ("k", wk_ap), ("v", wv_ap)):
                w = singles.tile([128, NDC, D], fp16, tag=f"w{name}")
                nc.gpsimd.dma_start(w[:], ap.rearrange("(c p) j -> p c j", p=128))
                w_sb[name] = w
            b_sb = {}
            for name, ap in (("q", bq_ap), ("k", bk_ap), ("v", bv_ap)):
                t = singles.tile([D, 1], fp32, tag=f"b{name}")
                nc.gpsimd.dma_start(t[:], ap[:])
                b_sb[name] = t
            xTs = []
            for dc in range(NDC):
                t = singles.tile([128, S], fp16, tag=f"xT{dc}",
                                 name=f"xT{dc}")
                xTs.append(t)
            # Head pieces land first so QT-sb0/sb1 and KT-sb0/1 can start
            # immediately; later pieces chained behind so the heads get the
            # bandwidth. head1 descriptors issue from the scalar engine in
            # parallel with sync issuing the rest.
            head1, head2, tail, hi_d = [], [], [], []
            for dc in range(NDC):
                head1.append(nc.scalar.dma_start(
                    xTs[dc][:, :512],
                    xT_ap[dc * 128:(dc + 1) * 128, :512]))
            for dc in range(NDC):
                head2.append(nc.sync.dma_start(
                    xTs[dc][:, 512:1024],
                    xT_ap[dc * 128:(dc + 1) * 128, 512:1024]))
            for dc in range(NDC):
                tail.append(nc.sync.dma_start(
                    xTs[dc][:, 1024:SK],
                    xT_ap[dc * 128:(dc + 1) * 128, 1024:SK]))
            for dc in range(NDC):
                hi_d.append(nc.sync.dma_start(
                    xTs[dc][:, SK:],
                    xT_ap[dc * 128:(dc + 1) * 128, SK:]))
            for dc in range(NDC):
                add_dep_helper(tail[dc].ins, head2[dc].ins,
                               reason="xT lo tails yield bandwidth to heads")
                add_dep_helper(hi_d[dc].ins, tail[dc].ins,
                               reason="xT hi halves yield bandwidth to lo")
            identity = singles.tile([128, 128], fp16)
            make_identity(nc, identity[:])

            QT = acts.tile([D, S], fp16, tag="QT")
            KT = acts.tile([D, SK], fp16, tag="KT")
            VT = acts.tile([D, SK], fp16, tag="VT")
            V = acts.tile([128, NKK, D], fp16, tag="V")
            Vs = acts.tile([128, NKK, D], fp16, tag="Vs")
            # S-A (2048-wide) writes slot 0 only (kk=0: slots 0,1), S-B
            # writes slots 2,3; zero-fill so the rsum reduce over all 4
            # slots is correct for every kk.
            sums = acts.tile([128, NKK, 4], fp32, tag="sums")
            nc.vector.memset(sums[:], 0.0)
            rsum = acts.tile([128, NKK], fp32, tag="rsum")
            rrec = acts.tile([128, NKK], fp32, tag="rrec")
            sum0b = acts.tile([128, 1], fp32, tag="sum0b")
            # exp(scores) for q 0:2048 (written by S-A) stays resident for
            # the trailing PV pass; q 2048:4096 rotates through a small pool
            # consumed by the PV pipelined inside S-B.
            expT_A = acts.tile([128, NKK, S // 2], fp16, tag="expT_A")

            # ---------------- Phases -------------------------------------
            # P1-head: QT-sb0 + KT-sb0 (gated only on the first head DMA
            #     pieces) so exp starts as early as possible.
            # S-A: scores+exp for q 0:2048 into the persistent expT_A at
            #     [128,2048] exp granularity (the PV pool isn't open yet so
            #     scores can take 4 PSUM banks x 2 buffers = all 8). All
            #     remaining projection work runs as self-contained units,
            #     one per S-A step, allocating transient accumulators from
            #     the same rotating pool.
            # S-B: scores+exp for q 2048:4096 at [128,1024] granularity
            #     (4 banks) + PV for q 2048:4096 accumulating in the other
            #     4 banks, software-pipelined one kk behind.
            # O2: PV for q 0:2048 from expT_A.
            def pv_matmuls(kk, po, src_tile, src_off):
                for qb in range(4):
                    nc.tensor.matmul(
                        po[qb][:], Vs[:, kk, :],
                        src_tile[:, src_off + qb * 512:
                                 src_off + (qb + 1) * 512],
                        start=(kk == 0), stop=(kk == NKK - 1))

            def drain_po(po, qb_base):
                for qb in range(4):
                    ob = outp.tile([D, 512], fp16, tag="ob")
                    nc.vector.tensor_copy(ob[:], po[qb][:])
                    nc.sync.dma_start(
                        outT_ap[:, (qb_base + qb) * 512:
                                (qb_base + qb + 1) * 512], ob[:])

            with (
                tc.tile_pool(name="ps_sA", bufs=2, space="PSUM") as ps_sA,
                tc.tile_pool(name="ps_proj", bufs=4, space="PSUM") as ps_proj,
            ):

                def proj_block(wname, dst, bias, sb):
                    acc = ps_proj.tile([D, 512], fp32, tag="pp",
                                       name=f"a{wname}{sb}")
                    for dc in range(NDC):
                        nc.tensor.matmul(
                            acc[:], w_sb[wname][:, dc, :],
                            xTs[dc][:, sb * 512:(sb + 1) * 512],
                            start=(dc == 0), stop=(dc == NDC - 1))
                    nc.vector.tensor_scalar_add(
                        dst[:, sb * 512:(sb + 1) * 512], acc[:], bias[:])

                def v_trans4(kk4):
                    pt = ps_proj.tile([128, 4, D], fp16, tag="pp",
                                      name=f"pt{kk4}")
                    for k in range(4):
                        nc.tensor.transpose(
                            pt[:, k, :],
                            VT[:, (kk4 + k) * 128:(kk4 + k + 1) * 128],
                            identity[:D, :D])
                    nc.vector.tensor_copy(V[:, kk4:kk4 + 4, :], pt[:])

                # P1 head: only what the first scores need.
                proj_block("q", QT, b_sb["q"], 0)
                proj_block("k", KT, b_sb["k"], 0)

                # Deferred units, one per S-A step, ordered by DMA arrival:
                # lo tails (~20us) before hi halves (~28us); V-transposes
                # (no DMA dependency) last.
                units = [
                    lambda: proj_block("k", KT, b_sb["k"], 2),
                    lambda: proj_block("k", KT, b_sb["k"], 3),
                    lambda: proj_block("q", QT, b_sb["q"], 2),
                    lambda: proj_block("q", QT, b_sb["q"], 3),
                    lambda: proj_block("v", VT, b_sb["v"], 0),
                    lambda: proj_block("v", VT, b_sb["v"], 1),
                    lambda: proj_block("v", VT, b_sb["v"], 2),
                    lambda: proj_block("v", VT, b_sb["v"], 3),
                    lambda: proj_block("q", QT, b_sb["q"], 4),
                    lambda: proj_block("q", QT, b_sb["q"], 5),
                    lambda: proj_block("q", QT, b_sb["q"], 6),
                    lambda: proj_block("q", QT, b_sb["q"], 7),
                    lambda: v_trans4(0),
                    lambda: v_trans4(4),
                    lambda: v_trans4(8),
                    lambda: v_trans4(12),
                ]

                # S-A: qq-outer so the whole first pass (q 0:1024) runs off
                # the head DMA pieces alone.
                ui = 0
                for qq in range(2):
                    for kk in range(NKK):
                        if qq == 0 and kk == 0:
                            # Finer first steps off the first head piece,
                            # second-head projections interleaved.
                            for j, acc in ((0, sums[:, 0, 0:1]),
                                           (1, sum0b[:])):
                                ps = ps_sA.tile([128, 512], fp32,
                                                tag="ps", name=f"ps0{j}")
                                nc.tensor.matmul(
                                    ps[:], KT[:, :128],
                                    QT[:, j * 512:(j + 1) * 512],
                                    start=True, stop=True)
                                nc.scalar.activation(
                                    expT_A[:, 0, j * 512:(j + 1) * 512],
                                    ps[:],
                                    mybir.ActivationFunctionType.Exp,
                                    accum_out=acc)
                                if j == 0:
                                    proj_block("q", QT, b_sb["q"], 1)
                                    proj_block("k", KT, b_sb["k"], 1)
                            continue
                        ps = ps_sA.tile([128, 1024], fp32, tag="ps")
                        for j in range(2):
                            nc.tensor.matmul(
                                ps[:, j * 512:(j + 1) * 512],
                                KT[:, kk * 128:(kk + 1) * 128],
                                QT[:, qq * 1024 + j * 512:
                                   qq * 1024 + (j + 1) * 512],
                                start=True, stop=True)
                        nc.scalar.activation(
                            expT_A[:, kk, qq * 1024:(qq + 1) * 1024],
                            ps[:], mybir.ActivationFunctionType.Exp,
                            accum_out=sums[:, kk, qq:qq + 1])
                        if ui < len(units):
                            units[ui]()
                            ui += 1
                while ui < len(units):
                    units[ui]()
                    ui += 1

            # S-B: scores+exp for q 2048:4096 + pipelined PV(q hi half).
            with (
                tc.tile_pool(name="ps_sB", bufs=2, space="PSUM") as ps_sB,
                tc.tile_pool(name="ps_o1", bufs=4, space="PSUM") as ps_o1,
                tc.tile_pool(name="exphi", bufs=2) as exphi_pool,
            ):
                po1 = [ps_o1.tile([D, 512], fp32, tag="po",
                                  name=f"po1_{i}") for i in range(4)]
                prev_hi = None
                for kk in range(NKK):
                    exp_hi = exphi_pool.tile([128, S // 2], fp16,
                                             tag="exp_hi")
                    for qq in (2, 3):
                        ps = ps_sB.tile([128, 1024], fp32, tag="psb")
                        for j in range(2):
                            nc.tensor.matmul(
                                ps[:, j * 512:(j + 1) * 512],
                                KT[:, kk * 128:(kk + 1) * 128],
                                QT[:, qq * 1024 + j * 512:
                                   qq * 1024 + (j + 1) * 512],
                                start=True, stop=True)
                        nc.scalar.activation(
                            exp_hi[:, (qq - 2) * 1024:(qq - 1) * 1024],
                            ps[:], mybir.ActivationFunctionType.Exp,
                            accum_out=sums[:, kk, qq:qq + 1])
                    nc.vector.reduce_sum(rsum[:, kk:kk + 1],
                                         sums[:, kk, :],
                                         axis=mybir.AxisListType.X)
                    if kk == 0:
                        nc.vector.tensor_add(rsum[:, 0:1], rsum[:, 0:1],
                                             sum0b[:])
                    nc.vector.reciprocal(rrec[:, kk:kk + 1],
                                         rsum[:, kk:kk + 1])
                    nc.vector.tensor_scalar_mul(Vs[:, kk, :], V[:, kk, :],
                                                rrec[:, kk:kk + 1])
                    if kk > 0:
                        pv_matmuls(kk - 1, po1, prev_hi[:], 0)
                    prev_hi = exp_hi
                pv_matmuls(NKK - 1, po1, prev_hi[:], 0)
                drain_po(po1, 4)

            # O2: PV for q 0:2048 from the persistent expT_A.
            with tc.tile_pool(name="ps_o2", bufs=4, space="PSUM") as ps_o2:
                po2 = [ps_o2.tile([D, 512], fp32, tag="po2",
                                  name=f"po2_{i}") for i in range(4)]
                for kk in range(NKK - 1):
                    pv_matmuls(kk, po2, expT_A[:, kk, :], 0)
                for qb in range(4):
                    nc.tensor.matmul(
                        po2[qb][:], Vs[:, NKK - 1, :],
                        expT_A[:, NKK - 1, qb * 512:(qb + 1) * 512],
                        start=False, stop=True)
                    ob = outp.tile([D, 512], fp16, tag="ob")
                    (nc.vector.tensor_copy if qb % 2 == 0
                     else nc.scalar.copy)(ob[:], po2[qb][:])
                    nc.sync.dma_start(
                        outT_ap[:, qb * 512:(qb + 1) * 512], ob[:])

    _dedup_ldweights(nc, mybir)
    nc.compile()
    return nc


def _dedup_ldweights(nc, mybir):
    """Drop InstLdweights that reload the weights already resident in the PE
    array (identical source AP as the previous load, with only
    non-self-loading matmuls in between). Tile's lowering emits one
    LDWEIGHTS per matmul; consecutive matmuls sharing a stationary operand
    only need the first."""
    remap = {}
    removed = 0
    for fn in nc.m.functions:
        for bb in fn.blocks:
            keep = []
            last_sig = None
            last_kept = None
            for inst in bb.instructions:
                if isinstance(inst, mybir.InstLdweights):
                    w = inst.ins[0]
                    try:
                        sig = (str(w.memref), str(w.memsetref), w.offset,
                               str(w.ap), str(w.dtype),
                               inst.perf_mode, inst.is_transpose)
                    except Exception:
                        sig = None
                    if sig is not None and last_kept is not None \
                            and sig == last_sig:
                        remap[inst.name] = last_kept.name
                        del nc.inst_map[inst.name]
                        removed += 1
                        continue
                    last_sig = sig
                    last_kept = inst
                elif isinstance(inst, mybir.InstMatmult):
                    if inst.ldweights is not False:
                        last_sig = None
                        last_kept = None
                keep.append(inst)
            if len(keep) != len(bb.instructions):
                bb.instructions[:] = keep
    if remap:
        for fn in nc.m.functions:
            for bb in fn.blocks:
                for inst in bb.instructions:
                    inst.remap_dependency_names(remap)
    return removed


def _get_module():
    if "nc" not in _CACHE:
        _CACHE["nc"] = _build_module()
    return _CACHE["nc"]


def kernel(x, Wq, bq, Wk, bk, Wv, bv, _trace=False):
    from concourse.bass_utils import run_bass_kernel_spmd

    x = np.asarray(x, dtype=np.float32)
    Wq = np.asarray(Wq, dtype=np.float32)
    bq = np.asarray(bq, dtype=np.float32)
    Wk = np.asarray(Wk, dtype=np.float32)
    bk = np.asarray(bk, dtype=np.float32)
    Wv = np.asarray(Wv, dtype=np.float32)
    bv = np.asarray(bv, dtype=np.float32)

    nc = _get_module()

    scale = np.float32(1.0 / np.sqrt(D))
    wq16 = (Wq * scale).astype(np.float16)
    wk16 = Wk.astype(np.float16)
    wv16 = Wv.astype(np.float16)
    bq_s = (bq * scale).astype(np.float32).reshape(D, 1)
    bk_s = bk.astype(np.float32).reshape(D, 1)
    bv_s = bv.astype(np.float32).reshape(D, 1)

    in_maps = []
    for c in range(N_CORES):
        b, kh = divmod(c, 2)
        xb = x[b]
        if kh:
            xb = np.concatenate([xb[SK:], xb[:SK]], axis=0)
        in_maps.append({
            "xT": np.ascontiguousarray(xb.T).astype(np.float16),
            "wq": wq16, "wk": wk16, "wv": wv16,
            "bq": bq_s, "bk": bk_s, "bv": bv_s,
        })

    res = run_bass_kernel_spmd(nc, in_maps,
                               core_ids=list(range(N_CORES)), trace=_trace)

    out = np.zeros((B, S, D), dtype=np.float32)
    for c in range(N_CORES):
        b, kh = divmod(c, 2)
        o = res.results[c]["outT"].T.astype(np.float32)  # [S, D], rolled q-order
        if kh:
            o = np.concatenate([o[SK:], o[:SK]], axis=0)
        out[b] += o
    if _trace:
        kernel.last_exec_time_ns = res.exec_time_ns
        kernel.last_result = res
    return out


# revision 24
# speedup vs baseline: 1.0364x; 1.0345x over previous
"""Trainium2 Bass kernel for single-head attention with softmax over the query axis.

Reference computation (B=4, S=4096, DIM=768, D=96):
    q = x @ Wq + bq; k = x @ Wk + bk; v = x @ Wv + bv        # [B,S,D]
    att = einsum('bqd,bkd->bqk', q, k) / sqrt(D)             # [B,Sq,Sk]
    p   = softmax(att, axis=1)                               # over the QUERY axis
    out = einsum('bqk,bkd->bqd', p, v)

Sharding: 8 cores = 4 batches x 2 key-halves. Softmax over q is local to a
key-shard (it normalizes each key-column over all queries), and the output
contraction over k is a sum over the two key-halves, done host-side.

SPMD uniformity trick: every core runs the identical program "K/V come from
rows 0:2048 of my x, Q from all 4096 rows". The host hands core (b, kh=1) a
row-rolled copy of x[b] so its key half lands in rows 0:2048; softmax over q
is permutation-invariant, and the host un-rolls that core's partial output.

Host precomputation (legal data prep inside kernel()): x is rolled,
transposed to xT [768, 4096] and cast to fp16; Wq/bq are pre-scaled by
1/sqrt(D) so no separate score scaling is needed; weights pre-cast to fp16.

On-device (per core):
  xT  [768, 4096] fp16 in SBUF (12 KB/partition)
  QT = Wq^T xT  [96, 4096], KT/VT likewise for rows 0:2048      (fp16)
  V[kk]  [128, 96] = PE-transpose of VT 128-column blocks        (fp16)
  scoresT[128k, q] = KT_kk^T QT in PSUM; exp on scalar engine with fused
      row-sum (accum_out); no max-subtraction (scores bounded ~|7.3|)
  Vs[kk] = V[kk] * (1/rowsum_kk)  folds softmax normalization into V
  outT[96, 4096] += Vs_kk^T @ expT_kk, accumulated in PSUM over kk;
      PV for q-blocks 0:2048 is software-pipelined inside the scores/exp
      loop (PSUM: 4 banks scores + 4 banks PV), the rest follows after.
"""

import os
import sys

import numpy as np

for _p in ("/opt/trn_rl_repo",):
    if _p not in sys.path and os.path.isdir(_p):
        sys.path.insert(0, _p)

B, S, DIM, D = 4, 4096, 768, 96
SK = S // 2          # local keys per core
N_CORES = 8
NDC = DIM // 128     # 6 dim chunks
NKK = SK // 128      # 16 local key chunks

_CACHE = {}


def _build_module():
    import concourse.bass as bass
    import concourse.tile as tile
    from concourse import bacc, mybir
    from concourse.masks import make_identity
    from concourse.tile import add_dep_helper

    fp32 = mybir.dt.float32
    fp16 = mybir.dt.float16

    nc = bacc.Bacc("TRN2", target_bir_lowering=False, debug=False,
                   num_devices=N_CORES)

    xT_ap = nc.dram_tensor("xT", [DIM, S], fp16, kind="ExternalInput").ap()
    wq_ap = nc.dram_tensor("wq", [DIM, D], fp16, kind="ExternalInput").ap()
    wk_ap = nc.dram_tensor("wk", [DIM, D], fp16, kind="ExternalInput").ap()
    wv_ap = nc.dram_tensor("wv", [DIM, D], fp16, kind="ExternalInput").ap()
    bq_ap = nc.dram_tensor("bq", [D, 1], fp32, kind="ExternalInput").ap()
    bk_ap = nc.dram_tensor("bk", [D, 1], fp32, kind="ExternalInput").ap()
    bv_ap = nc.dram_tensor("bv", [D, 1], fp32, kind="ExternalInput").ap()
    outT_ap = nc.dram_tensor("outT", [D, S], fp16, kind="ExternalOutput").ap()

    with tile.TileContext(nc) as tc:
        with (
            tc.tile_pool(name="singles", bufs=1) as singles,
            tc.tile_pool(name="acts", bufs=1) as acts,
            tc.tile_pool(name="outp", bufs=4) as outp,
        ):
            # Weights/biases first (tiny), then xT halves chained two-deep
            # so early chunks finish early (concurrent DMAs share bandwidth
            # equally; unchained, the first chunk lands no earlier than the
            # last and compute can't start for ~17us).
            w_sb = {}
            for name, ap in (("q", wq_ap), ("k", wk_ap), ("v", wv_ap)):
                w = singles.tile([128, NDC, D], fp16, tag=f"w{name}")
                nc.gpsimd.dma_start(w[:], ap.rearrange("(c p) j -> p c j", p=128))
                w_sb[name] = w
            b_sb = {}
            for name, ap in (("q", bq_ap), ("k", bk_ap), ("v", bv_ap)):
                t = singles.tile([D, 1], fp32, tag=f"b{name}")
                nc.gpsimd.dma_start(t[:], ap[:])
                b_sb[name] = t
            xTs = []
            for dc in range(NDC):
                t = singles.tile([128, S], fp16, tag=f"xT{dc}",
                                 name=f"xT{dc}")
                xTs.append(t)
            # Head pieces land first so QT-sb0/sb1 and KT-sb0/1 can start
            # immediately; later pieces chained behind so the heads get the
            # bandwidth. head1 descriptors issue from the scalar engine in
            # parallel with sync issuing the rest.
            head1, head2, tail, hi_d = [], [], [], []
            for dc in range(NDC):
                head1.append(nc.scalar.dma_start(
                    xTs[dc][:, :512],
                    xT_ap[dc * 128:(dc + 1) * 128, :512]))
            for dc in range(NDC):
                head2.append(nc.sync.dma_start(
                    xTs[dc][:, 512:1024],
                    xT_ap[dc * 128:(dc + 1) * 128, 512:1024]))
            for dc in range(NDC):
                tail.append(nc.sync.dma_start(
                    xTs[dc][:, 1024:SK],
                    xT_ap[dc * 128:(dc + 1) * 128, 1024:SK]))
            for dc in range(NDC):
                hi_d.append(nc.sync.dma_start(
                    xTs[dc][:, SK:],
                    xT_ap[dc * 128:(dc + 1) * 128, SK:]))
            for dc in range(NDC):
                add_dep_helper(tail[dc].ins, head2[dc].ins,
                               reason="xT lo tails yield bandwidth to heads")
                add_dep_helper(hi_d[dc].ins, tail[dc].ins,
                               reason="xT hi halves yield bandwidth to lo")
            identity = singles.tile([128, 128], fp16)
            make_identity(nc, identity[:])

            QT = acts.tile([D, S], fp16, tag="QT")
            KT = acts.tile([D, SK], fp16, tag="KT")
            VT = acts.tile([D, SK], fp16, tag="VT")
            V = acts.tile([128, NKK, D], fp16, tag="V")
            Vs = acts.tile([128, NKK, D], fp16, tag="Vs")
            # S-A (2048-wide) writes slot 0 only (kk=0: slots 0,1), S-B
            # writes slots 2,3; zero-fill so the rsum reduce over all 4
            # slots is correct for every kk.
            sums = acts.tile([128, NKK, 4], fp32, tag="sums")
            nc.vector.memset(sums[:], 0.0)
            rsum = acts.tile([128, NKK], fp32, tag="rsum")
            rrec = acts.tile([128, NKK], fp32, tag="rrec")
            sum0b = acts.tile([128, 1], fp32, tag="sum0b")
            # exp(scores) for q 0:2048 (written by S-A) stays resident for
            # the trailing PV pass; q 2048:4096 rotates through a small pool
            # consumed by the PV pipelined inside S-B.
            expT_A = acts.tile([128, NKK, S // 2], fp16, tag="expT_A")

            # ---------------- Phases -------------------------------------
            # P1-head: QT-sb0 + KT-sb0 (gated only on the first head DMA
            #     pieces) so exp starts as early as possible.
            # S-A: scores+exp for q 0:2048 into the persistent expT_A at
            #     [128,2048] exp granularity (the PV pool isn't open yet so
            #     scores can take 4 PSUM banks x 2 buffers = all 8). All
            #     remaining projection work runs as self-contained units,
            #     one per S-A step, allocating transient accumulators from
            #     the same rotating pool.
            # S-B: scores+exp for q 2048:4096 at [128,1024] granularity
            #     (4 banks) + PV for q 2048:4096 accumulating in the other
            #     4 banks, software-pipelined one kk behind.
            # O2: PV for q 0:2048 from expT_A.
            def pv_matmuls(kk, po, src_tile, src_off):
                for qb in range(4):
                    nc.tensor.matmul(
                        po[qb][:], Vs[:, kk, :],
                        src_tile[:, src_off + qb * 512:
                                 src_off + (qb + 1) * 512],
                        start=(kk == 0), stop=(kk == NKK - 1))

            def drain_po(po, qb_base):
                for qb in range(4):
                    ob = outp.tile([D, 512], fp16, tag="ob")
                    nc.vector.tensor_copy(ob[:], po[qb][:])
                    nc.sync.dma_start(
                        outT_ap[:, (qb_base + qb) * 512:
                                (qb_base + qb + 1) * 512], ob[:])

            with (
                tc.tile_pool(name="ps_sA", bufs=2, space="PSUM") as ps_sA,
                tc.tile_pool(name="ps_proj", bufs=4, space="PSUM") as ps_proj,
            ):

                def proj_block(wname, dst, bias, sb):
                    acc = ps_proj.tile([D, 512], fp32, tag="pp",
                                       name=f"a{wname}{sb}")
                    for dc in range(NDC):
                        nc.tensor.matmul(
                            acc[:], w_sb[wname][:, dc, :],
                            xTs[dc][:, sb * 512:(sb + 1) * 512],
                            start=(dc == 0), stop=(dc == NDC - 1))
                    nc.vector.tensor_scalar_add(
                        dst[:, sb * 512:(sb + 1) * 512], acc[:], bias[:])

                def v_trans4(kk4):
                    pt = ps_proj.tile([128, 4, D], fp16, tag="pp",
                                      name=f"pt{kk4}")
                    for k in range(4):
                        nc.tensor.transpose(
                            pt[:, k, :],
                            VT[:, (kk4 + k) * 128:(kk4 + k + 1) * 128],
                            identity[:D, :D])
                    nc.vector.tensor_copy(V[:, kk4:kk4 + 4, :], pt[:])

                # P1 head: only what the first scores need.
                proj_block("q", QT, b_sb["q"], 0)
                proj_block("k", KT, b_sb["k"], 0)

                # Deferred units, one per S-A step, ordered by DMA arrival:
                # lo tails (~20us) before hi halves (~28us); V-transposes
                # (no DMA dependency) last. VT/QT-hi run dc-outer with 4
                # accumulators so each weight chunk loads once (the
                # LDWEIGHTS dedup pass collapses the repeats); accumulators
                # allocate lazily so pool-slot order matches first use.
                lazy = {}

                def accs4(key):
                    if key not in lazy:
                        lazy[key] = [ps_proj.tile([D, 512], fp32, tag="pp",
                                                  name=f"{key}{i}")
                                     for i in range(4)]
                    return lazy[key]

                def pass_mms(key, wname, sb_base, dc):
                    accs = accs4(key)
                    for i in range(4):
                        sb = sb_base + i
                        nc.tensor.matmul(
                            accs[i][:], w_sb[wname][:, dc, :],
                            xTs[dc][:, sb * 512:(sb + 1) * 512],
                            start=(dc == 0), stop=(dc == NDC - 1))

                def pass_drain(key, dst, sb_base, bias):
                    accs = accs4(key)
                    for i in range(4):
                        sb = sb_base + i
                        nc.vector.tensor_scalar_add(
                            dst[:, sb * 512:(sb + 1) * 512],
                            accs[i][:], bias[:])

                units = [
                    lambda: proj_block("k", KT, b_sb["k"], 2),
                    lambda: proj_block("k", KT, b_sb["k"], 3),
                    lambda: proj_block("q", QT, b_sb["q"], 2),
                    lambda: proj_block("q", QT, b_sb["q"], 3),
                ]
                for dc in range(NDC):
                    units.append(lambda dc=dc: pass_mms("avt", "v", 0, dc))
                units.append(lambda: pass_drain("avt", VT, 0, b_sb["v"]))
                for dc in range(NDC):
                    units.append(lambda dc=dc: pass_mms("aqh", "q", 4, dc))
                units.append(lambda: pass_drain("aqh", QT, 4, b_sb["q"]))
                units += [
                    lambda: v_trans4(0),
                    lambda: v_trans4(4),
                    lambda: v_trans4(8),
                    lambda: v_trans4(12),
                ]

                # S-A: qq-outer so the whole first pass (q 0:1024) runs off
                # the head DMA pieces alone.
                ui = 0
                for qq in range(2):
                    for kk in range(NKK):
                        if qq == 0 and kk == 0:
                            # Finer first steps off the first head piece,
                            # second-head projections interleaved.
                            for j, acc in ((0, sums[:, 0, 0:1]),
                                           (1, sum0b[:])):
                                ps = ps_sA.tile([128, 512], fp32,
                                                tag="ps", name=f"ps0{j}")
                                nc.tensor.matmul(
                                    ps[:], KT[:, :128],
                                    QT[:, j * 512:(j + 1) * 512],
                                    start=True, stop=True)
                                nc.scalar.activation(
                                    expT_A[:, 0, j * 512:(j + 1) * 512],
                                    ps[:],
                                    mybir.ActivationFunctionType.Exp,
                                    accum_out=acc)
                                if j == 0:
                                    proj_block("q", QT, b_sb["q"], 1)
                                    proj_block("k", KT, b_sb["k"], 1)
                            continue
                        ps = ps_sA.tile([128, 1024], fp32, tag="ps")
                        for j in range(2):
                            nc.tensor.matmul(
                                ps[:, j * 512:(j + 1) * 512],
                                KT[:, kk * 128:(kk + 1) * 128],
                                QT[:, qq * 1024 + j * 512:
                                   qq * 1024 + (j + 1) * 512],
                                start=True, stop=True)
                        nc.scalar.activation(
                            expT_A[:, kk, qq * 1024:(qq + 1) * 1024],
                            ps[:], mybir.ActivationFunctionType.Exp,
                            accum_out=sums[:, kk, qq:qq + 1])
                        if ui < len(units):
                            units[ui]()
                            ui += 1
                while ui < len(units):
                    units[ui]()
                    ui += 1

            # S-B: scores+exp for q 2048:4096 + pipelined PV(q hi half).
            with (
                tc.tile_pool(name="ps_sB", bufs=2, space="PSUM") as ps_sB,
                tc.tile_pool(name="ps_o1", bufs=4, space="PSUM") as ps_o1,
                tc.tile_pool(name="exphi", bufs=2) as exphi_pool,
            ):
                po1 = [ps_o1.tile([D, 512], fp32, tag="po",
                                  name=f"po1_{i}") for i in range(4)]
                prev_hi = None
                for kk in range(NKK):
                    exp_hi = exphi_pool.tile([128, S // 2], fp16,
                                             tag="exp_hi")
                    for qq in (2, 3):
                        ps = ps_sB.tile([128, 1024], fp32, tag="psb")
                        for j in range(2):
                            nc.tensor.matmul(
                                ps[:, j * 512:(j + 1) * 512],
                                KT[:, kk * 128:(kk + 1) * 128],
                                QT[:, qq * 1024 + j * 512:
                                   qq * 1024 + (j + 1) * 512],
                                start=True, stop=True)
                        nc.scalar.activation(
                            exp_hi[:, (qq - 2) * 1024:(qq - 1) * 1024],
                            ps[:], mybir.ActivationFunctionType.Exp,
                            accum_out=sums[:, kk, qq:qq + 1])
                    nc.vector.reduce_sum(rsum[:, kk:kk + 1],
                                         sums[:, kk, :],
                                         axis=mybir.AxisListType.X)
                    if kk == 0:
                        nc.vector.tensor_add(rsum[:, 0:1], rsum[:, 0:1],
                                             sum0b[:])
                    nc.vector.reciprocal(rrec[:, kk:kk + 1],
                                         rsum[:, kk:kk + 1])
                    nc.vector.tensor_scalar_mul(Vs[:, kk, :], V[:, kk, :],
                                                rrec[:, kk:kk + 1])
                    if kk > 0:
                        pv_matmuls(kk - 1, po1, prev_hi[:], 0)
                    prev_hi = exp_hi
                pv_matmuls(NKK - 1, po1, prev_hi[:], 0)
                drain_po(po1, 4)

            # O2: PV for q 0:2048 from the persistent expT_A.
            with tc.tile_pool(name="ps_o2", bufs=4, space="PSUM") as ps_o2:
                po2 = [ps_o2.tile([D, 512], fp32, tag="po2",
                                  name=f"po2_{i}") for i in range(4)]
                for kk in range(NKK - 1):
                    pv_matmuls(kk, po2, expT_A[:, kk, :], 0)
                for qb in range(4):
                    nc.tensor.matmul(
                        po2[qb][:], Vs[:, NKK - 1, :],
                        expT_A[:, NKK - 1, qb * 512:(qb + 1) * 512],
                        start=False, stop=True)
                    ob = outp.tile([D, 512], fp16, tag="ob")
                    (nc.vector.tensor_copy if qb % 2 == 0
                     else nc.scalar.copy)(ob[:], po2[qb][:])
                    nc.sync.dma_start(
                        outT_ap[:, qb * 512:(qb + 1) * 512], ob[:])

    _dedup_ldweights(nc, mybir)
    nc.compile()
    return nc


def _dedup_ldweights(nc, mybir):
    """Drop InstLdweights that reload the weights already resident in the PE
    array (identical source AP as the previous load, with only
    non-self-loading matmuls in between). Tile's lowering emits one
    LDWEIGHTS per matmul; consecutive matmuls sharing a stationary operand
    only need the first."""
    remap = {}
    removed = 0
    for fn in nc.m.functions:
        for bb in fn.blocks:
            keep = []
            last_sig = None
            last_kept = None
            for inst in bb.instructions:
                if isinstance(inst, mybir.InstLdweights):
                    w = inst.ins[0]
                    try:
                        sig = (str(w.memref), str(w.memsetref), w.offset,
                               str(w.ap), str(w.dtype),
                               inst.perf_mode, inst.is_transpose)
                    except Exception:
                        sig = None
                    if sig is not None and last_kept is not None \
                            and sig == last_sig:
                        remap[inst.name] = last_kept.name
                        del nc.inst_map[inst.name]
                        removed += 1
                        continue
                    last_sig = sig
                    last_kept = inst
                elif isinstance(inst, mybir.InstMatmult):
                    if inst.ldweights is not False:
                        last_sig = None
                        last_kept = None
                keep.append(inst)
            if len(keep) != len(bb.instructions):
                bb.instructions[:] = keep
    if remap:
        for fn in nc.m.functions:
            for bb in fn.blocks:
                for inst in bb.instructions:
                    inst.remap_dependency_names(remap)
    return removed


def _get_module():
    if "nc" not in _CACHE:
        _CACHE["nc"] = _build_module()
    return _CACHE["nc"]


def kernel(x, Wq, bq, Wk, bk, Wv, bv, _trace=False):
    from concourse.bass_utils import run_bass_kernel_spmd

    x = np.asarray(x, dtype=np.float32)
    Wq = np.asarray(Wq, dtype=np.float32)
    bq = np.asarray(bq, dtype=np.float32)
    Wk = np.asarray(Wk, dtype=np.float32)
    bk = np.asarray(bk, dtype=np.float32)
    Wv = np.asarray(Wv, dtype=np.float32)
    bv = np.asarray(bv, dtype=np.float32)

    nc = _get_module()

    scale = np.float32(1.0 / np.sqrt(D))
    wq16 = (Wq * scale).astype(np.float16)
    wk16 = Wk.astype(np.float16)
    wv16 = Wv.astype(np.float16)
    bq_s = (bq * scale).astype(np.float32).reshape(D, 1)
    bk_s = bk.astype(np.float32).reshape(D, 1)
    bv_s = bv.astype(np.float32).reshape(D, 1)

    in_maps = []
    for c in range(N_CORES):
        b, kh = divmod(c, 2)
        xb = x[b]
        if kh:
            xb = np.concatenate([xb[SK:], xb[:SK]], axis=0)
        in_maps.append({
            "xT": np.ascontiguousarray(xb.T).astype(np.float16),
            "wq": wq16, "wk": wk16, "wv": wv16,
            "bq": bq_s, "bk": bk_s, "bv": bv_s,
        })

    res = run_bass_kernel_spmd(nc, in_maps,
                               core_ids=list(range(N_CORES)), trace=_trace)

    out = np.zeros((B, S, D), dtype=np.float32)
    for c in range(N_CORES):
        b, kh = divmod(c, 2)
        o = res.results[c]["outT"].T.astype(np.float32)  # [S, D], rolled q-order
        if kh:
            o = np.concatenate([o[SK:], o[:SK]], axis=0)
        out[b] += o
    if _trace:
        kernel.last_exec_time_ns = res.exec_time_ns
        kernel.last_result = res
    return out


# revision 25
# speedup vs baseline: 1.0540x; 1.0170x over previous
"""Trainium2 Bass kernel for single-head attention with softmax over the query axis.

Reference computation (B=4, S=4096, DIM=768, D=96):
    q = x @ Wq + bq; k = x @ Wk + bk; v = x @ Wv + bv        # [B,S,D]
    att = einsum('bqd,bkd->bqk', q, k) / sqrt(D)             # [B,Sq,Sk]
    p   = softmax(att, axis=1)                               # over the QUERY axis
    out = einsum('bqk,bkd->bqd', p, v)

Sharding: 8 cores = 4 batches x 2 key-halves. Softmax over q is local to a
key-shard (it normalizes each key-column over all queries), and the output
contraction over k is a sum over the two key-halves, done host-side.

SPMD uniformity trick: every core runs the identical program "K/V come from
rows 0:2048 of my x, Q from all 4096 rows". The host hands core (b, kh=1) a
row-rolled copy of x[b] so its key half lands in rows 0:2048; softmax over q
is permutation-invariant, and the host un-rolls that core's partial output.

Host precomputation (legal data prep inside kernel()): x is rolled,
transposed to xT [768, 4096] and cast to fp16; Wq/bq are pre-scaled by
1/sqrt(D) so no separate score scaling is needed; weights pre-cast to fp16.

On-device (per core):
  xT  [768, 4096] fp16 in SBUF (12 KB/partition)
  QT = Wq^T xT  [96, 4096], KT/VT likewise for rows 0:2048      (fp16)
  V[kk]  [128, 96] = PE-transpose of VT 128-column blocks        (fp16)
  scoresT[128k, q] = KT_kk^T QT in PSUM; exp on scalar engine with fused
      row-sum (accum_out); no max-subtraction (scores bounded ~|7.3|)
  Vs[kk] = V[kk] * (1/rowsum_kk)  folds softmax normalization into V
  outT[96, 4096] += Vs_kk^T @ expT_kk, accumulated in PSUM over kk;
      PV for q-blocks 0:2048 is software-pipelined inside the scores/exp
      loop (PSUM: 4 banks scores + 4 banks PV), the rest follows after.
"""

import os
import sys

import numpy as np

for _p in ("/opt/trn_rl_repo",):
    if _p not in sys.path and os.path.isdir(_p):
        sys.path.insert(0, _p)

B, S, DIM, D = 4, 4096, 768, 96
SK = S // 2          # local keys per core
N_CORES = 8
NDC = DIM // 128     # 6 dim chunks
NKK = SK // 128      # 16 local key chunks

_CACHE = {}


def _build_module():
    import concourse.bass as bass
    import concourse.tile as tile
    from concourse import bacc, mybir
    from concourse.masks import make_identity
    from concourse.tile import add_dep_helper

    fp32 = mybir.dt.float32
    fp16 = mybir.dt.float16

    nc = bacc.Bacc("TRN2", target_bir_lowering=False, debug=False,
                   num_devices=N_CORES)

    xT_ap = nc.dram_tensor("xT", [DIM, S], fp16, kind="ExternalInput").ap()
    wq_ap = nc.dram_tensor("wq", [DIM, D], fp16, kind="ExternalInput").ap()
    wk_ap = nc.dram_tensor("wk", [DIM, D], fp16, kind="ExternalInput").ap()
    wv_ap = nc.dram_tensor("wv", [DIM, D], fp16, kind="ExternalInput").ap()
    bq_ap = nc.dram_tensor("bq", [D, 1], fp32, kind="ExternalInput").ap()
    bk_ap = nc.dram_tensor("bk", [D, 1], fp32, kind="ExternalInput").ap()
    bv_ap = nc.dram_tensor("bv", [D, 1], fp32, kind="ExternalInput").ap()
    outT_ap = nc.dram_tensor("outT", [D, S], fp16, kind="ExternalOutput").ap()

    with tile.TileContext(nc) as tc:
        with (
            tc.tile_pool(name="singles", bufs=1) as singles,
            tc.tile_pool(name="acts", bufs=1) as acts,
            tc.tile_pool(name="outp", bufs=4) as outp,
        ):
            # Weights/biases first (tiny), then xT halves chained two-deep
            # so early chunks finish early (concurrent DMAs share bandwidth
            # equally; unchained, the first chunk lands no earlier than the
            # last and compute can't start for ~17us).
            w_sb = {}
            for name, ap in (("q", wq_ap), ("k", wk_ap), ("v", wv_ap)):
                w = singles.tile([128, NDC, D], fp16, tag=f"w{name}")
                nc.gpsimd.dma_start(w[:], ap.rearrange("(c p) j -> p c j", p=128))
                w_sb[name] = w
            b_sb = {}
            for name, ap in (("q", bq_ap), ("k", bk_ap), ("v", bv_ap)):
                t = singles.tile([D, 1], fp32, tag=f"b{name}")
                nc.gpsimd.dma_start(t[:], ap[:])
                b_sb[name] = t
            xTs = []
            for dc in range(NDC):
                t = singles.tile([128, S], fp16, tag=f"xT{dc}",
                                 name=f"xT{dc}")
                xTs.append(t)
            # Head pieces land first so QT-sb0/sb1 and KT-sb0/1 can start
            # immediately; later pieces chained behind so the heads get the
            # bandwidth. head1 descriptors issue from the scalar engine in
            # parallel with sync issuing the rest.
            head1, head2, tail, hi_d = [], [], [], []
            for dc in range(NDC):
                head1.append(nc.scalar.dma_start(
                    xTs[dc][:, :512],
                    xT_ap[dc * 128:(dc + 1) * 128, :512]))
            for dc in range(NDC):
                head2.append(nc.sync.dma_start(
                    xTs[dc][:, 512:1024],
                    xT_ap[dc * 128:(dc + 1) * 128, 512:1024]))
            for dc in range(NDC):
                tail.append(nc.sync.dma_start(
                    xTs[dc][:, 1024:SK],
                    xT_ap[dc * 128:(dc + 1) * 128, 1024:SK]))
            for dc in range(NDC):
                hi_d.append(nc.sync.dma_start(
                    xTs[dc][:, SK:],
                    xT_ap[dc * 128:(dc + 1) * 128, SK:]))
            for dc in range(NDC):
                add_dep_helper(tail[dc].ins, head2[dc].ins,
                               reason="xT lo tails yield bandwidth to heads")
                add_dep_helper(hi_d[dc].ins, tail[dc].ins,
                               reason="xT hi halves yield bandwidth to lo")
            identity = singles.tile([128, 128], fp16)
            make_identity(nc, identity[:])
            # Dummy exp during the DMA wait: walrus places the ~2.7us
            # ACT_TABLE_LOAD before the first ACTIVATE, so trigger it while
            # the scalar engine is otherwise idle instead of on the
            # critical path of the first real exp.
            warm_in = singles.tile([128, 8], fp32, tag="warm_in")
            nc.vector.memset(warm_in[:], 0.0)
            warm_out = singles.tile([128, 8], fp32, tag="warm_out")
            nc.scalar.activation(warm_out[:], warm_in[:],
                                 mybir.ActivationFunctionType.Exp)

            QT = acts.tile([D, S], fp16, tag="QT")
            KT = acts.tile([D, SK], fp16, tag="KT")
            VT = acts.tile([D, SK], fp16, tag="VT")
            V = acts.tile([128, NKK, D], fp16, tag="V")
            Vs = acts.tile([128, NKK, D], fp16, tag="Vs")
            # S-A (2048-wide) writes slot 0 only (kk=0: slots 0,1), S-B
            # writes slots 2,3; zero-fill so the rsum reduce over all 4
            # slots is correct for every kk.
            sums = acts.tile([128, NKK, 4], fp32, tag="sums")
            nc.vector.memset(sums[:], 0.0)
            rsum = acts.tile([128, NKK], fp32, tag="rsum")
            rrec = acts.tile([128, NKK], fp32, tag="rrec")
            sum0b = acts.tile([128, 1], fp32, tag="sum0b")
            # exp(scores) for q 0:2048 (written by S-A) stays resident for
            # the trailing PV pass; q 2048:4096 rotates through a small pool
            # consumed by the PV pipelined inside S-B.
            expT_A = acts.tile([128, NKK, S // 2], fp16, tag="expT_A")

            # ---------------- Phases -------------------------------------
            # P1-head: QT-sb0 + KT-sb0 (gated only on the first head DMA
            #     pieces) so exp starts as early as possible.
            # S-A: scores+exp for q 0:2048 into the persistent expT_A at
            #     [128,2048] exp granularity (the PV pool isn't open yet so
            #     scores can take 4 PSUM banks x 2 buffers = all 8). All
            #     remaining projection work runs as self-contained units,
            #     one per S-A step, allocating transient accumulators from
            #     the same rotating pool.
            # S-B: scores+exp for q 2048:4096 at [128,1024] granularity
            #     (4 banks) + PV for q 2048:4096 accumulating in the other
            #     4 banks, software-pipelined one kk behind.
            # O2: PV for q 0:2048 from expT_A.
            def pv_matmuls(kk, po, src_tile, src_off):
                for qb in range(4):
                    nc.tensor.matmul(
                        po[qb][:], Vs[:, kk, :],
                        src_tile[:, src_off + qb * 512:
                                 src_off + (qb + 1) * 512],
                        start=(kk == 0), stop=(kk == NKK - 1))

            def drain_po(po, qb_base):
                for qb in range(4):
                    ob = outp.tile([D, 512], fp16, tag="ob")
                    nc.vector.tensor_copy(ob[:], po[qb][:])
                    nc.sync.dma_start(
                        outT_ap[:, (qb_base + qb) * 512:
                                (qb_base + qb + 1) * 512], ob[:])

            with (
                tc.tile_pool(name="ps_sA", bufs=2, space="PSUM") as ps_sA,
                tc.tile_pool(name="ps_proj", bufs=4, space="PSUM") as ps_proj,
            ):

                def proj_block(wname, dst, bias, sb):
                    acc = ps_proj.tile([D, 512], fp32, tag="pp",
                                       name=f"a{wname}{sb}")
                    for dc in range(NDC):
                        nc.tensor.matmul(
                            acc[:], w_sb[wname][:, dc, :],
                            xTs[dc][:, sb * 512:(sb + 1) * 512],
                            start=(dc == 0), stop=(dc == NDC - 1))
                    nc.vector.tensor_scalar_add(
                        dst[:, sb * 512:(sb + 1) * 512], acc[:], bias[:])

                def v_trans4(kk4):
                    pt = ps_proj.tile([128, 4, D], fp16, tag="pp",
                                      name=f"pt{kk4}")
                    for k in range(4):
                        nc.tensor.transpose(
                            pt[:, k, :],
                            VT[:, (kk4 + k) * 128:(kk4 + k + 1) * 128],
                            identity[:D, :D])
                    nc.vector.tensor_copy(V[:, kk4:kk4 + 4, :], pt[:])

                # P1 head: only what the first scores need.
                proj_block("q", QT, b_sb["q"], 0)
                proj_block("k", KT, b_sb["k"], 0)

                # Deferred units, one per S-A step, ordered by DMA arrival:
                # lo tails (~20us) before hi halves (~28us); V-transposes
                # (no DMA dependency) last. VT/QT-hi run dc-outer with 4
                # accumulators so each weight chunk loads once (the
                # LDWEIGHTS dedup pass collapses the repeats); accumulators
                # allocate lazily so pool-slot order matches first use.
                lazy = {}

                def accs4(key):
                    if key not in lazy:
                        lazy[key] = [ps_proj.tile([D, 512], fp32, tag="pp",
                                                  name=f"{key}{i}")
                                     for i in range(4)]
                    return lazy[key]

                def pass_mms(key, wname, sb_base, dc):
                    accs = accs4(key)
                    for i in range(4):
                        sb = sb_base + i
                        nc.tensor.matmul(
                            accs[i][:], w_sb[wname][:, dc, :],
                            xTs[dc][:, sb * 512:(sb + 1) * 512],
                            start=(dc == 0), stop=(dc == NDC - 1))

                def pass_drain(key, dst, sb_base, bias):
                    accs = accs4(key)
                    for i in range(4):
                        sb = sb_base + i
                        nc.vector.tensor_scalar_add(
                            dst[:, sb * 512:(sb + 1) * 512],
                            accs[i][:], bias[:])

                units = [
                    lambda: proj_block("k", KT, b_sb["k"], 2),
                    lambda: proj_block("k", KT, b_sb["k"], 3),
                    lambda: proj_block("q", QT, b_sb["q"], 2),
                    lambda: proj_block("q", QT, b_sb["q"], 3),
                ]
                for dc in range(NDC):
                    units.append(lambda dc=dc: pass_mms("avt", "v", 0, dc))
                units.append(lambda: pass_drain("avt", VT, 0, b_sb["v"]))
                for dc in range(NDC):
                    units.append(lambda dc=dc: pass_mms("aqh", "q", 4, dc))
                units.append(lambda: pass_drain("aqh", QT, 4, b_sb["q"]))
                units += [
                    lambda: v_trans4(0),
                    lambda: v_trans4(4),
                    lambda: v_trans4(8),
                    lambda: v_trans4(12),
                ]

                # S-A: qq-outer so the whole first pass (q 0:1024) runs off
                # the head DMA pieces alone.
                ui = 0
                for qq in range(2):
                    for kk in range(NKK):
                        if qq == 0 and kk == 0:
                            # Finer first steps off the first head piece,
                            # second-head projections interleaved.
                            for j, acc in ((0, sums[:, 0, 0:1]),
                                           (1, sum0b[:])):
                                ps = ps_sA.tile([128, 512], fp32,
                                                tag="ps", name=f"ps0{j}")
                                nc.tensor.matmul(
                                    ps[:], KT[:, :128],
                                    QT[:, j * 512:(j + 1) * 512],
                                    start=True, stop=True)
                                nc.scalar.activation(
                                    expT_A[:, 0, j * 512:(j + 1) * 512],
                                    ps[:],
                                    mybir.ActivationFunctionType.Exp,
                                    accum_out=acc)
                                if j == 0:
                                    proj_block("q", QT, b_sb["q"], 1)
                                    proj_block("k", KT, b_sb["k"], 1)
                            continue
                        ps = ps_sA.tile([128, 1024], fp32, tag="ps")
                        for j in range(2):
                            nc.tensor.matmul(
                                ps[:, j * 512:(j + 1) * 512],
                                KT[:, kk * 128:(kk + 1) * 128],
                                QT[:, qq * 1024 + j * 512:
                                   qq * 1024 + (j + 1) * 512],
                                start=True, stop=True)
                        nc.scalar.activation(
                            expT_A[:, kk, qq * 1024:(qq + 1) * 1024],
                            ps[:], mybir.ActivationFunctionType.Exp,
                            accum_out=sums[:, kk, qq:qq + 1])
                        if ui < len(units):
                            units[ui]()
                            ui += 1
                while ui < len(units):
                    units[ui]()
                    ui += 1

            # S-B: scores+exp for q 2048:4096 + pipelined PV(q hi half).
            with (
                tc.tile_pool(name="ps_sB", bufs=2, space="PSUM") as ps_sB,
                tc.tile_pool(name="ps_o1", bufs=4, space="PSUM") as ps_o1,
                tc.tile_pool(name="exphi", bufs=2) as exphi_pool,
            ):
                po1 = [ps_o1.tile([D, 512], fp32, tag="po",
                                  name=f"po1_{i}") for i in range(4)]
                prev_hi = None
                for kk in range(NKK):
                    exp_hi = exphi_pool.tile([128, S // 2], fp16,
                                             tag="exp_hi")
                    for qq in (2, 3):
                        ps = ps_sB.tile([128, 1024], fp32, tag="psb")
                        for j in range(2):
                            nc.tensor.matmul(
                                ps[:, j * 512:(j + 1) * 512],
                                KT[:, kk * 128:(kk + 1) * 128],
                                QT[:, qq * 1024 + j * 512:
                                   qq * 1024 + (j + 1) * 512],
                                start=True, stop=True)
                        nc.scalar.activation(
                            exp_hi[:, (qq - 2) * 1024:(qq - 1) * 1024],
                            ps[:], mybir.ActivationFunctionType.Exp,
                            accum_out=sums[:, kk, qq:qq + 1])
                    nc.vector.reduce_sum(rsum[:, kk:kk + 1],
                                         sums[:, kk, :],
                                         axis=mybir.AxisListType.X)
                    if kk == 0:
                        nc.vector.tensor_add(rsum[:, 0:1], rsum[:, 0:1],
                                             sum0b[:])
                    nc.vector.reciprocal(rrec[:, kk:kk + 1],
                                         rsum[:, kk:kk + 1])
                    nc.vector.tensor_scalar_mul(Vs[:, kk, :], V[:, kk, :],
                                                rrec[:, kk:kk + 1])
                    if kk > 0:
                        pv_matmuls(kk - 1, po1, prev_hi[:], 0)
                    prev_hi = exp_hi
                pv_matmuls(NKK - 1, po1, prev_hi[:], 0)
                drain_po(po1, 4)

            # O2: PV for q 0:2048 from the persistent expT_A.
            with tc.tile_pool(name="ps_o2", bufs=4, space="PSUM") as ps_o2:
                po2 = [ps_o2.tile([D, 512], fp32, tag="po2",
                                  name=f"po2_{i}") for i in range(4)]
                for kk in range(NKK - 1):
                    pv_matmuls(kk, po2, expT_A[:, kk, :], 0)
                for qb in range(4):
                    nc.tensor.matmul(
                        po2[qb][:], Vs[:, NKK - 1, :],
                        expT_A[:, NKK - 1, qb * 512:(qb + 1) * 512],
                        start=False, stop=True)
                    ob = outp.tile([D, 512], fp16, tag="ob")
                    (nc.vector.tensor_copy if qb % 2 == 0
                     else nc.scalar.copy)(ob[:], po2[qb][:])
                    nc.sync.dma_start(
                        outT_ap[:, qb * 512:(qb + 1) * 512], ob[:])

    _dedup_ldweights(nc, mybir)
    nc.compile()
    return nc


def _dedup_ldweights(nc, mybir):
    """Drop InstLdweights that reload the weights already resident in the PE
    array (identical source AP as the previous load, with only
    non-self-loading matmuls in between). Tile's lowering emits one
    LDWEIGHTS per matmul; consecutive matmuls sharing a stationary operand
    only need the first."""
    remap = {}
    removed = 0
    for fn in nc.m.functions:
        for bb in fn.blocks:
            keep = []
            last_sig = None
            last_kept = None
            for inst in bb.instructions:
                if isinstance(inst, mybir.InstLdweights):
                    w = inst.ins[0]
                    try:
                        sig = (str(w.memref), str(w.memsetref), w.offset,
                               str(w.ap), str(w.dtype),
                               inst.perf_mode, inst.is_transpose)
                    except Exception:
                        sig = None
                    if sig is not None and last_kept is not None \
                            and sig == last_sig:
                        remap[inst.name] = last_kept.name
                        del nc.inst_map[inst.name]
                        removed += 1
                        continue
                    last_sig = sig
                    last_kept = inst
                elif isinstance(inst, mybir.InstMatmult):
                    if inst.ldweights is not False:
                        last_sig = None
                        last_kept = None
                keep.append(inst)
            if len(keep) != len(bb.instructions):
                bb.instructions[:] = keep
    if remap:
        for fn in nc.m.functions:
            for bb in fn.blocks:
                for inst in bb.instructions:
                    inst.remap_dependency_names(remap)
    return removed


def _get_module():
    if "nc" not in _CACHE:
        _CACHE["nc"] = _build_module()
    return _CACHE["nc"]


def kernel(x, Wq, bq, Wk, bk, Wv, bv, _trace=False):
    from concourse.bass_utils import run_bass_kernel_spmd

    x = np.asarray(x, dtype=np.float32)
    Wq = np.asarray(Wq, dtype=np.float32)
    bq = np.asarray(bq, dtype=np.float32)
    Wk = np.asarray(Wk, dtype=np.float32)
    bk = np.asarray(bk, dtype=np.float32)
    Wv = np.asarray(Wv, dtype=np.float32)
    bv = np.asarray(bv, dtype=np.float32)

    nc = _get_module()

    scale = np.float32(1.0 / np.sqrt(D))
    wq16 = (Wq * scale).astype(np.float16)
    wk16 = Wk.astype(np.float16)
    wv16 = Wv.astype(np.float16)
    bq_s = (bq * scale).astype(np.float32).reshape(D, 1)
    bk_s = bk.astype(np.float32).reshape(D, 1)
    bv_s = bv.astype(np.float32).reshape(D, 1)

    in_maps = []
    for c in range(N_CORES):
        b, kh = divmod(c, 2)
        xb = x[b]
        if kh:
            xb = np.concatenate([xb[SK:], xb[:SK]], axis=0)
        in_maps.append({
            "xT": np.ascontiguousarray(xb.T).astype(np.float16),
            "wq": wq16, "wk": wk16, "wv": wv16,
            "bq": bq_s, "bk": bk_s, "bv": bv_s,
        })

    res = run_bass_kernel_spmd(nc, in_maps,
                               core_ids=list(range(N_CORES)), trace=_trace)

    out = np.zeros((B, S, D), dtype=np.float32)
    for c in range(N_CORES):
        b, kh = divmod(c, 2)
        o = res.results[c]["outT"].T.astype(np.float32)  # [S, D], rolled q-order
        if kh:
            o = np.concatenate([o[SK:], o[:SK]], axis=0)
        out[b] += o
    if _trace:
        kernel.last_exec_time_ns = res.exec_time_ns
        kernel.last_result = res
    return out
